# revision 7
# baseline (speedup 1.0000x reference)
"""Trainium2 Bass kernel for AttentionalPoolerWMasking (v2).

Computation (see reference):
  xk = LN(x) over CTX_DIM; q = LN(query) over D_MODEL
  bias = log(clamp(size)) + attention_mask                    [B, L]
  qh = (q @ Wq.T + bq) * 1/sqrt(hd)                           [Q, D]
  kh = xk @ Wk.T ; vh = xk @ Wv.T + bv                        [B, L, D]
  scores = qh @ kh.T + bias ; attn = softmax(scores, L)       per head
  out = (attn @ vh) @ Wo.T + bo                               [B, Q, D]

v2 changes over the bf16 baseline:
 - query path (LN + Wq + bq + scale) computed on host in f32; qhT is a
   plain input. bk is dropped entirely: it shifts scores by a per-(q,h)
   constant, and softmax is shift-invariant.
 - K projection runs in fp8e4m3 DoubleRow (2x PE): stationary
   wk8 [128, 2, 96] pairs adjacent c-blocks, moving xn8 [128, 2, 512].
   Scales: xn8 = 8*xn, wk8 = 256*(lnkw o Wk); 1/2048 folded into the
   PSUM->SBUF kh copy. V path stays bf16 (accuracy).
 - LN stats use host-precomputed fp8 tensors: x8r = e4m3(8*x) and
   x28 = e4m3(x^2); mean and sumsq are DoubleRow matmuls with a ones
   stationary (was: bf16 ones matmul + on-device scalar squares).
 - ln_k affine folds (w into Wv/Wk, b into bv) done on host.
Strategy: data-parallel over B across 8 cores (4 batches/core);
software pipelining as baseline: front_end(b+1) between projections(b)
and attention(b).
"""

import sys

sys.path.insert(0, "/opt/trn_rl_repo")

import numpy as np

import concourse.bass as bass
import concourse.mybir as mybir
import concourse.tile as tile
from concourse import bacc, bass_utils

F32 = mybir.dt.float32
BF16 = mybir.dt.bfloat16
FP8 = mybir.dt.float8e4
DR = mybir.MatmulPerfMode.DoubleRow
AF = mybir.ActivationFunctionType
OP = mybir.AluOpType

B, L, C = 32, 1024, 1024          # x: [B, L, C]
D, H, HD, Q = 768, 8, 96, 256     # d_model, heads, head dim, queries
EPS = 1e-5
N_CORES = 8
BL = B // N_CORES                 # batches per core
SCALE = 1.0 / float(np.sqrt(HD))

CB = C // 128                     # 8 c-blocks (contraction of projections)
CBP = CB // 2                     # 4 paired c-blocks for DoubleRow
LB = L // 128                     # 8 l-blocks
QB = Q // 128                     # 2 q-blocks

SX = 8.0                          # scale baked into xn8 and x8r
SW = 256.0                        # scale baked into wk8
KH_DESCALE = 1.0 / (SX * SW)


def build_program():
    nc = bacc.Bacc("TRN2", target_bir_lowering=False, debug=False,
                   num_devices=N_CORES)

    # ---- DRAM I/O ----
    xT = nc.dram_tensor("xT", [BL, C, L], BF16, kind="ExternalInput").ap()
    x8r_d = nc.dram_tensor("x8r", [BL, CB, 128, L], FP8,
                           kind="ExternalInput").ap()
    x28_d = nc.dram_tensor("x28", [BL, CB, 128, L], FP8,
                           kind="ExternalInput").ap()
    szmk_d = nc.dram_tensor("szmk", [BL, 128, 2 * LB], F32,
                            kind="ExternalInput").ap()
    qhT_d = nc.dram_tensor("qhT_hm", [HD, H, Q], BF16,
                           kind="ExternalInput").ap()
    wk8_d = nc.dram_tensor("Wk8", [128, CBP, 2, D], FP8,
                           kind="ExternalInput").ap()
    wvT_d = nc.dram_tensor("WvT", [C, D], BF16, kind="ExternalInput").ap()
    woT_d = nc.dram_tensor("WoT", [HD, H, D], BF16, kind="ExternalInput").ap()
    bv_d = nc.dram_tensor("bv_f", [D], F32, kind="ExternalInput").ap()
    bo_d = nc.dram_tensor("bo", [D], F32, kind="ExternalInput").ap()
    out_d = nc.dram_tensor("out", [BL, Q, D], F32, kind="ExternalOutput").ap()

    def bcast_dram(ap1d, p, n):
        return bass.AP(tensor=ap1d.tensor, offset=ap1d.offset,
                       ap=[[0, p], [1, n]])

    from contextlib import ExitStack
    with tile.TileContext(nc) as tc, ExitStack() as es:
        const = es.enter_context(tc.tile_pool(name="const", bufs=1))

        # PSUM pools: kv 3 + sc 3 + av 2 = 8 banks
        kvps = es.enter_context(tc.tile_pool(name="kvps", bufs=3, space="PSUM"))
        scps = es.enter_context(tc.tile_pool(name="scps", bufs=3, space="PSUM"))
        avps = es.enter_context(tc.tile_pool(name="avps", bufs=2, space="PSUM"))
        fips = scps

        # batch-0/1 x loads go first so the PE front-end starts early and
        # the GpSimd queue never parks ahead of a pending load
        xnp = es.enter_context(tc.tile_pool(name="xnp", bufs=2))
        xns = [None] * BL
        for bb in range(2):
            xns[bb] = xnp.tile([128, CB, L], BF16, tag="xn", name=f"xn_b{bb}")
            for cb in range(CB):
                nc.gpsimd.dma_start(out=xns[bb][:, cb, :],
                                    in_=xT[bb, cb * 128:(cb + 1) * 128, :])

        # fp8 stats streams (batch 0 synchronously; b+1 prefetched in
        # front_end(b))
        x8p = es.enter_context(tc.tile_pool(name="x8p", bufs=1))
        x2p = es.enter_context(tc.tile_pool(name="x2p", bufs=1))
        x8s = [None] * BL
        x2s = [None] * BL
        x8s[0] = x8p.tile([128, CB, L], FP8, tag="x8r", name="x8r_b0")
        nc.sync.dma_start(out=x8s[0], in_=x8r_d[0].rearrange("a p l -> p a l"))
        x2s[0] = x2p.tile([128, CB, L], FP8, tag="x28", name="x28_b0")
        nc.sync.dma_start(out=x2s[0], in_=x28_d[0].rearrange("a p l -> p a l"))

        # ---- persistent constants ----
        wv = const.tile([128, CB, D], BF16, tag="wv")
        nc.gpsimd.dma_start(out=wv, in_=wvT_d.rearrange("(a p) d -> p a d", p=128))
        wk8 = const.tile([128, CBP, 2, D], FP8, tag="wk8")
        nc.gpsimd.dma_start(out=wk8, in_=wk8_d)
        wo = const.tile([HD, H, D], BF16, tag="wo")
        nc.gpsimd.dma_start(out=wo, in_=woT_d)
        qhT = const.tile([HD, H, Q], BF16, tag="qhT")
        nc.sync.dma_start(out=qhT, in_=qhT_d)

        bvb = const.tile([128, D], F32, tag="bvb")
        nc.gpsimd.dma_start(out=bvb, in_=bcast_dram(bv_d, 128, D))
        bob = const.tile([128, D], F32, tag="bob")
        nc.gpsimd.dma_start(out=bob, in_=bcast_dram(bo_d, 128, D))
        ones64 = const.tile([128, LB * H], F32, tag="ones64")
        nc.vector.memset(ones64, 1.0)
        ones8t = const.tile([128, 2, 16], FP8, tag="ones8")
        nc.vector.memset(ones8t, 1.0)
        ones8 = ones8t[:, :, 0:1]
        eps8 = const.tile([128, 1], F32, tag="eps8")
        nc.vector.memset(eps8, EPS)

        # front-end pools
        rows = es.enter_context(tc.tile_pool(name="rows", bufs=1))
        bcastp = es.enter_context(tc.tile_pool(name="bcastp", bufs=1))
        biasp = es.enter_context(tc.tile_pool(name="biasp", bufs=2))
        xn8p = es.enter_context(tc.tile_pool(name="xn8p", bufs=2))

        # attention-phase pools
        recipp = es.enter_context(tc.tile_pool(name="recipp", bufs=2))
        khp = es.enter_context(tc.tile_pool(name="khp", bufs=2))
        drp = es.enter_context(tc.tile_pool(name="drp", bufs=2, space="DRAM"))
        vhp = es.enter_context(tc.tile_pool(name="vhp", bufs=1))
        expp = es.enter_context(tc.tile_pool(name="expp", bufs=4))
        outtp = es.enter_context(tc.tile_pool(name="outtp", bufs=8))
        finp = es.enter_context(tc.tile_pool(name="finp", bufs=2))

        xn8s = [None] * BL

        def front_end(b):
            # bias row: log(clamp(size)) + mask, in [128, LB] layout
            szmk = biasp.tile([128, 2 * LB], F32, tag="szmk")
            nc.sync.dma_start(out=szmk, in_=szmk_d[b])
            sz, msk = szmk[:, :LB], szmk[:, LB:]
            # size_c = m*(size-1)+1 with m = (size >= 0.5): clamps <0.5 -> 1
            m8 = biasp.tile([128, LB], F32, tag="m8")
            nc.vector.tensor_scalar(m8, sz, 0.5, None, op0=OP.is_ge)
            nc.vector.tensor_scalar_add(sz, sz, -1.0)
            nc.vector.tensor_tensor(sz, sz, m8, op=OP.mult)
            nc.vector.tensor_scalar_add(sz, sz, 1.0)
            biasT = biasp.tile([128, LB], F32, tag="biasT")
            nc.scalar.activation(biasT, sz, AF.Ln)
            nc.vector.tensor_tensor(biasT, biasT, msk, op=OP.add)

            xn = xns[b]
            x8t, x2t = x8s[b], x2s[b]
            # row stats via fp8 DoubleRow matmuls: [1, 512] psum rows ->
            # bounce via DMA into [128, 8] tiles for 128-lane reciprocal.
            murow = rows.tile([1, L], F32, tag="murow")
            sqrow = rows.tile([1, L], F32, tag="sqrow")
            for half in range(2):
                sl = slice(half * 512, (half + 1) * 512)
                mean_ps = scps.tile([1, 512], F32, tag="sc", name="mean_ps")
                sq_ps = scps.tile([1, 512], F32, tag="sc", name="sq_ps")
                for cp in range(CBP):
                    nc.tensor.matmul(mean_ps, ones8,
                                     x8t[:, 2 * cp:2 * cp + 2, sl],
                                     start=(cp == 0), stop=(cp == CBP - 1),
                                     perf_mode=DR)
                for cp in range(CBP):
                    nc.tensor.matmul(sq_ps, ones8,
                                     x2t[:, 2 * cp:2 * cp + 2, sl],
                                     start=(cp == 0), stop=(cp == CBP - 1),
                                     perf_mode=DR)
                nc.vector.tensor_scalar_mul(murow[0:1, sl], mean_ps,
                                            1.0 / (C * SX))
                nc.vector.tensor_scalar_mul(sqrow[0:1, sl], sq_ps, 1.0 / C)
            mu8 = rows.tile([128, 8], F32, tag="mu8")
            nc.sync.dma_start(out=mu8, in_=murow)
            var8 = rows.tile([128, 8], F32, tag="var8")
            nc.sync.dma_start(out=var8, in_=sqrow)
            t8 = rows.tile([128, 8], F32, tag="t8")
            nc.vector.tensor_tensor(t8, mu8, mu8, op=OP.mult)
            nc.vector.tensor_tensor(var8, var8, t8, op=OP.subtract)
            nc.scalar.activation(var8, var8, AF.Sqrt, bias=eps8)  # std
            r8 = rows.tile([128, 8], BF16, tag="r8")
            r8f = rows.tile([128, 8], F32, tag="r8f")
            nc.vector.reciprocal(r8f, var8)
            nc.vector.tensor_copy(r8, r8f)
            s8 = rows.tile([128, 8], BF16, tag="s8")  # -mu*r
            nc.vector.tensor_tensor(t8, mu8, r8f, op=OP.mult)
            nc.vector.tensor_scalar_mul(t8, t8, -1.0)
            nc.vector.tensor_copy(s8, t8)
            rbrow = rows.tile([1, L], BF16, tag="rbrow")
            nc.sync.dma_start(out=rbrow, in_=r8)
            sbrow = rows.tile([1, L], BF16, tag="sbrow")
            nc.sync.dma_start(out=sbrow, in_=s8)
            rxb = bcastp.tile([128, L], BF16, tag="rxb")
            nc.gpsimd.partition_broadcast(rxb, rbrow)
            sxb = bcastp.tile([128, L], BF16, tag="sxb")
            nc.gpsimd.partition_broadcast(sxb, sbrow)

            # normalize in place (affine folded into W), then cast out the
            # fp8 copy (x8) used by the DoubleRow K projection
            xn8s[b] = xn8p.tile([128, CB, L], FP8, tag="xn8", name=f"xn8_b{b}")
            for cb in range(CB):
                nc.vector.tensor_tensor(xn[:, cb, :], xn[:, cb, :], rxb,
                                        op=OP.mult)
                nc.vector.tensor_tensor(xn[:, cb, :], xn[:, cb, :], sxb,
                                        op=OP.add)
                nc.vector.tensor_scalar_mul(xn8s[b][:, cb, :], xn[:, cb, :],
                                            SX)
            # lookahead loads last: buffer-recycle semaphore waits must not
            # park the queues ahead of this batch's broadcasts
            if b + 1 < BL:
                x8s[b + 1] = x8p.tile([128, CB, L], FP8, tag="x8r",
                                      name=f"x8r_b{b + 1}")
                nc.sync.dma_start(out=x8s[b + 1],
                                  in_=x8r_d[b + 1].rearrange("a p l -> p a l"))
                x2s[b + 1] = x2p.tile([128, CB, L], FP8, tag="x28",
                                      name=f"x28_b{b + 1}")
                nc.sync.dma_start(out=x2s[b + 1],
                                  in_=x28_d[b + 1].rearrange("a p l -> p a l"))
            if b + 2 < BL:
                xns[b + 2] = xnp.tile([128, CB, L], BF16, tag="xn",
                                      name=f"xn_b{b + 2}")
                for cb in range(CB):
                    nc.gpsimd.dma_start(
                        out=xns[b + 2][:, cb, :],
                        in_=xT[b + 2, cb * 128:(cb + 1) * 128, :])
            return biasT

        def projections(b):
            xn = xns[b]
            xn8 = xn8s[b]
            # K projection in fp8 DoubleRow; kh descaled on copy (no bk:
            # per-(q,h) score shifts cancel in softmax)
            kh = khp.tile([HD, H, L], BF16, tag="kh")
            for h in range(H):
                kps0 = kvps.tile([128, 512], F32, tag="kv", name="kps0")
                kps1 = kvps.tile([128, 512], F32, tag="kv", name="kps1")
                for cp in range(CBP):
                    st, sp = (cp == 0), (cp == CBP - 1)
                    nc.tensor.matmul(kps0[:HD, :],
                                     wk8[:, cp, :, h * HD:(h + 1) * HD],
                                     xn8[:, 2 * cp:2 * cp + 2, 0:512],
                                     start=st, stop=sp, perf_mode=DR)
                    nc.tensor.matmul(kps1[:HD, :],
                                     wk8[:, cp, :, h * HD:(h + 1) * HD],
                                     xn8[:, 2 * cp:2 * cp + 2, 512:1024],
                                     start=st, stop=sp, perf_mode=DR)
                nc.vector.tensor_scalar_mul(kh[:, h, 0:512], kps0[:HD, :],
                                            KH_DESCALE)
                nc.scalar.mul(kh[:, h, 512:1024], kps1[:HD, :], KH_DESCALE)

            vh = vhp.tile([128, LB, H, HD + 1], BF16, tag="vh")
            nc.vector.tensor_copy(
                vh[:, :, :, HD:HD + 1],
                ones64.rearrange("p (a b c) -> p a b c", a=LB, b=H))
            for lb in range(LB):
                for dc in range(2):
                    dsl = slice(dc * 4 * HD, (dc + 1) * 4 * HD)
                    vps = kvps.tile([128, 512], F32, tag="kv")
                    for cb in range(CB):
                        nc.tensor.matmul(vps[:, :4 * HD],
                                         xn[:, cb, lb * 128:(lb + 1) * 128],
                                         wv[:, cb, dsl],
                                         start=(cb == 0), stop=(cb == CB - 1))
                    nc.vector.tensor_tensor(
                        vh[:, lb, 4 * dc:4 * dc + 4, 0:HD],
                        vps[:, :4 * HD], bvb[:, dsl], op=OP.add)
            return kh, vh

        def attention(b, kh, vh, biasT):
            serow = recipp.tile([1, H * Q], F32, tag="serow", bufs=1)
            ots = [None] * H
            for hp in range(H // 2):
                h0, h1 = 2 * hp, 2 * hp + 1
                av0 = avps.tile([HD + 1, Q], F32, tag="av", name=f"av{h0}")
                av1 = avps.tile([HD + 1, Q], F32, tag="av", name=f"av{h1}")
                for lb in range(LB):
                    sc = scps.tile([128, 2, Q], F32, tag="sc")
                    nc.tensor.matmul(sc[:, 0, :],
                                     kh[:, h0, lb * 128:(lb + 1) * 128],
                                     qhT[:, h0, :], start=True, stop=True)
                    nc.tensor.matmul(sc[:, 1, :],
                                     kh[:, h1, lb * 128:(lb + 1) * 128],
                                     qhT[:, h1, :], start=True, stop=True)
                    ex = expp.tile([128, 2, Q], BF16, tag="ex")
                    nc.scalar.activation(ex, sc, AF.Exp,
                                         bias=biasT[:, lb:lb + 1])
                    nc.tensor.matmul(av0, vh[:, lb, h0, :], ex[:, 0, :],
                                     start=(lb == 0), stop=(lb == LB - 1))
                    nc.tensor.matmul(av1, vh[:, lb, h1, :], ex[:, 1, :],
                                     start=(lb == 0), stop=(lb == LB - 1))
                for h, av in ((h0, av0), (h1, av1)):
                    nc.vector.tensor_copy(serow[0:1, h * Q:(h + 1) * Q],
                                          av[HD:HD + 1, :])
                    ot = outtp.tile([HD, Q], BF16, tag="ot", name=f"ot{h}")
                    nc.scalar.copy(ot, av[0:HD, :])
                    ots[h] = ot
            se8 = recipp.tile([128, H * Q // 128], F32, tag="se8")
            nc.scalar.dma_start(out=se8, in_=serow)
            nc.vector.reciprocal(se8, se8)
            se8b = recipp.tile([128, H * Q // 128], BF16, tag="se8b")
            nc.vector.tensor_copy(se8b, se8)
            sed = drp.tile([H * Q], BF16, tag="sed")
            nc.scalar.dma_start(out=sed, in_=se8b)
            rball = recipp.tile([HD, H, Q], BF16, tag="rball", bufs=1)
            nc.scalar.dma_start(out=rball.rearrange("p a q -> p (a q)"),
                                in_=bcast_dram(sed, HD, H * Q))
            otbs = []
            for h in range(H):
                nc.vector.tensor_tensor(ots[h], ots[h], rball[:, h, :],
                                        op=OP.mult)
                otbs.append(ots[h])

            # out projection: final[q, dm] = sum_h outT_h.T @ WoT_h  (+bo)
            for qb in range(QB):
                fin = finp.tile([128, D], F32, tag="fin")
                for dc, dn in ((0, 512), (512, 256)):
                    fps = fips.tile([128, 2, Q], F32, tag="sc", name="fps")
                    fpsv = fps.rearrange("p a q -> p (a q)")
                    for h in range(H):
                        nc.tensor.matmul(fpsv[:, :dn],
                                         otbs[h][:, qb * 128:(qb + 1) * 128],
                                         wo[:, h, dc:dc + dn],
                                         start=(h == 0), stop=(h == H - 1))
                    nc.vector.tensor_tensor(fin[:, dc:dc + dn], fpsv[:, :dn],
                                            bob[:, dc:dc + dn], op=OP.add)
                nc.scalar.dma_start(out=out_d[b, qb * 128:(qb + 1) * 128, :],
                                     in_=fin)

        # ---- software-pipelined per-batch schedule ----
        bias_cur = front_end(0)
        for b in range(BL):
            kh, vh = projections(b)
            bias_next = front_end(b + 1) if b + 1 < BL else None
            attention(b, kh, vh, bias_cur)
            bias_cur = bias_next

    nc.compile()
    return nc


_CACHE = {}


def make_in_maps(inputs):
    import ml_dtypes
    bf16 = ml_dtypes.bfloat16
    f8 = ml_dtypes.float8_e4m3

    x = np.ascontiguousarray(inputs["x"], dtype=np.float32)
    size = np.asarray(inputs["size"], dtype=np.float32)
    mask = np.asarray(inputs["attention_mask"], dtype=np.float32)
    query = np.asarray(inputs["query"], dtype=np.float32)
    lnqw = np.asarray(inputs["ln_q_w"], np.float32)
    lnqb = np.asarray(inputs["ln_q_b"], np.float32)
    lnkw = np.asarray(inputs["ln_k_w"], np.float32)
    lnkb = np.asarray(inputs["ln_k_b"], np.float32)
    Wq = np.asarray(inputs["Wq"], np.float32)
    Wk = np.asarray(inputs["Wk"], np.float32)
    Wv = np.asarray(inputs["Wv"], np.float32)
    Wo = np.asarray(inputs["Wo"], np.float32)

    xb = x.astype(bf16)
    xT = np.ascontiguousarray(xb.transpose(0, 2, 1))       # [B, C, L] bf16
    xf = xb.astype(np.float32)
    # fp8 stats streams, layout [B, CB, 128, L] (c = cb*128 + p)
    x8r = np.ascontiguousarray(
        (xf.transpose(0, 2, 1) * SX).reshape(B, CB, 128, L).astype(f8))
    x28 = np.ascontiguousarray(
        (xf.transpose(0, 2, 1) ** 2).reshape(B, CB, 128, L).astype(f8))

    # host query path in f32: qhT[i, h, q] = ((LN(q) @ Wq.T + bq) * scale)
    qmu = query.mean(-1, keepdims=True)
    qvar = query.var(-1, keepdims=True)
    qln = (query - qmu) / np.sqrt(qvar + EPS) * lnqw + lnqb
    qh = (qln @ Wq.T + np.asarray(inputs["bq"], np.float32)) * SCALE  # [Q, D]
    qhT_hm = np.ascontiguousarray(
        qh.reshape(Q, H, HD).transpose(2, 1, 0).astype(bf16))  # [HD, H, Q]

    # folded K weights, fp8 with scale SW: [128, CBP, 2, D]
    wkf = (Wk * lnkw[None, :]).T                            # [C, D]
    Wk8 = np.ascontiguousarray(
        (wkf * SW).reshape(CBP, 2, 128, D).transpose(2, 0, 1, 3).astype(f8))
    # folded V weights + bias
    WvT = np.ascontiguousarray((Wv * lnkw[None, :]).T.astype(bf16))  # [C, D]
    bv_f = (np.asarray(inputs["bv"], np.float32) + lnkb @ Wv.T)
    WoT = np.ascontiguousarray(
        Wo.T.reshape(H, HD, D).transpose(1, 0, 2).astype(bf16))

    size2 = np.ascontiguousarray(size[:, :, 0])            # [B, L]
    mask2 = np.ascontiguousarray(mask[:, 0, :])            # [B, L]
    # size/mask combined, l = a*128 + p -> (b, p, a)
    szmk = np.ascontiguousarray(np.concatenate(
        [size2.reshape(B, LB, 128).transpose(0, 2, 1),
         mask2.reshape(B, LB, 128).transpose(0, 2, 1)], axis=2))

    common = {
        "qhT_hm": qhT_hm, "Wk8": Wk8, "WvT": WvT, "WoT": WoT,
        "bv_f": bv_f.astype(np.float32),
        "bo": np.asarray(inputs["bo"], np.float32),
    }
    in_maps = []
    for i in range(N_CORES):
        sl = slice(i * BL, (i + 1) * BL)
        m = dict(common)
        m["xT"] = np.ascontiguousarray(xT[sl])
        m["x8r"] = np.ascontiguousarray(x8r[sl])
        m["x28"] = np.ascontiguousarray(x28[sl])
        m["szmk"] = np.ascontiguousarray(szmk[sl])
        in_maps.append(m)

    return in_maps


def kernel(**inputs):
    in_maps = make_in_maps(inputs)
    if "nc" not in _CACHE:
        _CACHE["nc"] = build_program()
    nc = _CACHE["nc"]

    for attempt in range(3):
        res = bass_utils.run_bass_kernel_spmd(nc, in_maps,
                                              core_ids=list(range(N_CORES)))
        out = np.concatenate([res.results[i]["out"] for i in range(N_CORES)],
                             axis=0)
        if np.isfinite(out).all():
            return out
    return out


# revision 10
# speedup vs baseline: 1.1460x; 1.1460x over previous
"""Trainium2 Bass kernel for AttentionalPoolerWMasking (v3).

Computation (see reference):
  xk = LN(x) over CTX_DIM; q = LN(query) over D_MODEL
  bias = log(clamp(size)) + attention_mask                    [B, L]
  qh = (q @ Wq.T + bq) * 1/sqrt(hd)                           [Q, D]
  kh = xk @ Wk.T ; vh = xk @ Wv.T + bv                        [B, L, D]
  scores = qh @ kh.T + bias ; attn = softmax(scores, L)       per head
  out = (attn @ vh) @ Wo.T + bo                               [B, Q, D]

v3 design:
 - host (exact f32): query path -> qhT; ln_k folds (w into Wk/Wv, b into
   bv); bk dropped (per-(q,h) score shift, softmax-invariant); bv folded
   through the attention average into bo2 = bo + bv_f @ Wo.T (attn rows
   sum to 1 after the explicit 1/den divide); column sums of the folded
   weights for the LN mean correction; fp8 copies x8 = e4m3(8x) and
   x28 = e4m3(x^2) for the stats matmuls.
 - projections run on RAW x (no normalize pass at all):
     kh[d,l] = r[l]*(sum_c x8*wk8)*DESC + s[l]*colsumK[d]
     vh[l,d] = r[l]*(sum_c x*wv)       + s[l]*colsumV[d]
   with r = rsqrt(var+eps), s = -mu*r applied post-matmul (r,s in f32;
   row-broadcast for kh, per-partition columns for vh). K matmuls are
   fp8e4m3 DoubleRow (~1.65x PE); V stays bf16 for accuracy.
 - LN stats are DoubleRow matmuls on the host fp8 tensors; the whole
   stats->r/s chain runs OFF the PE critical path (only the PSUM->SBUF
   copies wait for it, matmuls never do).
 - software pipeline: fe(0) proj(0) fe(1) | att_core(b) proj(b+1)
   att_finish(b) fe(b+2): the softmax-reciprocal chain of batch b hides
   under projections(b+1).
Data-parallel over B across 8 cores (4 batches/core).
"""

import sys

sys.path.insert(0, "/opt/trn_rl_repo")

import numpy as np

import concourse.bass as bass
import concourse.mybir as mybir
import concourse.tile as tile
from concourse import bacc, bass_utils

F32 = mybir.dt.float32
BF16 = mybir.dt.bfloat16
FP8 = mybir.dt.float8e4
DR = mybir.MatmulPerfMode.DoubleRow
AF = mybir.ActivationFunctionType
OP = mybir.AluOpType

B, L, C = 32, 1024, 1024          # x: [B, L, C]
D, H, HD, Q = 768, 8, 96, 256     # d_model, heads, head dim, queries
EPS = 1e-5
N_CORES = 8
BL = B // N_CORES                 # batches per core
SCALE = 1.0 / float(np.sqrt(HD))

CB = C // 128                     # 8 c-blocks
CBP = CB // 2                     # 4 paired c-blocks for DoubleRow
LB = L // 128                     # 8 l-blocks
QB = Q // 128                     # 2 q-blocks

SX = 8.0                          # scale baked into x8
SW = 256.0                        # scale baked into wk8
KH_DESCALE = 1.0 / (SX * SW)


def build_program():
    nc = bacc.Bacc("TRN2", target_bir_lowering=False, debug=False,
                   num_devices=N_CORES)

    # ---- DRAM I/O ----
    xT = nc.dram_tensor("xT", [BL, C, L], BF16, kind="ExternalInput").ap()
    x8r_d = nc.dram_tensor("x8r", [BL, CB, 128, L], FP8,
                           kind="ExternalInput").ap()
    x28_d = nc.dram_tensor("x28", [BL, CB, 128, L], FP8,
                           kind="ExternalInput").ap()
    szmk_d = nc.dram_tensor("szmk", [BL, 128, 2 * LB], F32,
                            kind="ExternalInput").ap()
    qhT_d = nc.dram_tensor("qhT_hm", [HD, H, Q], BF16,
                           kind="ExternalInput").ap()
    wk8_d = nc.dram_tensor("Wk8", [128, CBP, 2, D], FP8,
                           kind="ExternalInput").ap()
    wvT_d = nc.dram_tensor("WvT", [C, D], BF16, kind="ExternalInput").ap()
    woT_d = nc.dram_tensor("WoT", [HD, H, D], BF16, kind="ExternalInput").ap()
    csk_d = nc.dram_tensor("csK_hm", [HD, H], F32, kind="ExternalInput").ap()
    csv_d = nc.dram_tensor("csV", [D], F32, kind="ExternalInput").ap()
    bo2_d = nc.dram_tensor("bo2", [D], F32, kind="ExternalInput").ap()
    out_d = nc.dram_tensor("out", [BL, Q, D], F32, kind="ExternalOutput").ap()

    def bcast_dram(ap1d, p, n):
        return bass.AP(tensor=ap1d.tensor, offset=ap1d.offset,
                       ap=[[0, p], [1, n]])

    from contextlib import ExitStack
    with tile.TileContext(nc) as tc, ExitStack() as es:
        const = es.enter_context(tc.tile_pool(name="const", bufs=1))

        # PSUM: kv 3 + sc 3 + av 2 = 8 banks
        kvps = es.enter_context(tc.tile_pool(name="kvps", bufs=3, space="PSUM"))
        scps = es.enter_context(tc.tile_pool(name="scps", bufs=3, space="PSUM"))
        avps = es.enter_context(tc.tile_pool(name="avps", bufs=2, space="PSUM"))
        fips = scps

        # batch-0/1 x loads go first so the PE front-end starts early
        xnp = es.enter_context(tc.tile_pool(name="xnp", bufs=2))
        xns = [None] * BL
        for bb in range(2):
            xns[bb] = xnp.tile([128, CB, L], BF16, tag="xn", name=f"xn_b{bb}")
            for cb in range(CB):
                nc.gpsimd.dma_start(out=xns[bb][:, cb, :],
                                    in_=xT[bb, cb * 128:(cb + 1) * 128, :])

        x8p = es.enter_context(tc.tile_pool(name="x8p", bufs=2))
        x2p = es.enter_context(tc.tile_pool(name="x2p", bufs=1))
        x8s = [None] * BL
        x2s = [None] * BL
        x8s[0] = x8p.tile([128, CB, L], FP8, tag="x8r", name="x8r_b0")
        nc.sync.dma_start(out=x8s[0], in_=x8r_d[0].rearrange("a p l -> p a l"))
        x2s[0] = x2p.tile([128, CB, L], FP8, tag="x28", name="x28_b0")
        nc.sync.dma_start(out=x2s[0], in_=x28_d[0].rearrange("a p l -> p a l"))

        # ---- persistent constants ----
        wv = const.tile([128, CB, D], BF16, tag="wv")
        nc.gpsimd.dma_start(out=wv, in_=wvT_d.rearrange("(a p) d -> p a d", p=128))
        wk8 = const.tile([128, CBP, 2, D], FP8, tag="wk8")
        nc.gpsimd.dma_start(out=wk8, in_=wk8_d)
        wo = const.tile([HD, H, D], BF16, tag="wo")
        nc.gpsimd.dma_start(out=wo, in_=woT_d)
        qhT = const.tile([HD, H, Q], BF16, tag="qhT")
        nc.sync.dma_start(out=qhT, in_=qhT_d)
        csk = const.tile([HD, H], F32, tag="csk")
        nc.sync.dma_start(out=csk, in_=csk_d)
        csvb = const.tile([128, D], F32, tag="csvb")
        nc.gpsimd.dma_start(out=csvb, in_=bcast_dram(csv_d, 128, D))
        bob = const.tile([128, D], F32, tag="bob")
        nc.gpsimd.dma_start(out=bob, in_=bcast_dram(bo2_d, 128, D))
        ones64 = const.tile([128, LB * H], F32, tag="ones64")
        nc.vector.memset(ones64, 1.0)
        ones8t = const.tile([128, 2, 16], FP8, tag="ones8")
        nc.vector.memset(ones8t, 1.0)
        ones8 = ones8t[:, :, 0:1]
        eps8 = const.tile([128, 1], F32, tag="eps8")
        nc.vector.memset(eps8, EPS)

        rows = es.enter_context(tc.tile_pool(name="rows", bufs=1))
        cols = es.enter_context(tc.tile_pool(name="cols", bufs=2))
        bcastp = es.enter_context(tc.tile_pool(name="bcastp", bufs=1))
        biasp = es.enter_context(tc.tile_pool(name="biasp", bufs=2))

        recipp = es.enter_context(tc.tile_pool(name="recipp", bufs=2))
        khp = es.enter_context(tc.tile_pool(name="khp", bufs=2))
        drp = es.enter_context(tc.tile_pool(name="drp", bufs=2, space="DRAM"))
        vhp = es.enter_context(tc.tile_pool(name="vhp", bufs=1))
        expp = es.enter_context(tc.tile_pool(name="expp", bufs=4))
        outtp = es.enter_context(tc.tile_pool(name="outtp", bufs=8))
        finp = es.enter_context(tc.tile_pool(name="finp", bufs=2))

        biasTs = [None] * (BL + 1)
        rxbs = [None] * BL
        sxbs = [None] * BL
        rvs = [None] * BL
        svs = [None] * BL

        def front_end(b):
            # bias row: log(clamp(size)) + mask, in [128, LB] layout
            szmk = biasp.tile([128, 2 * LB], F32, tag="szmk")
            nc.sync.dma_start(out=szmk, in_=szmk_d[b])
            sz, msk = szmk[:, :LB], szmk[:, LB:]
            m8 = biasp.tile([128, LB], F32, tag="m8")
            nc.vector.tensor_scalar(m8, sz, 0.5, None, op0=OP.is_ge)
            nc.vector.tensor_scalar_add(sz, sz, -1.0)
            nc.vector.tensor_tensor(sz, sz, m8, op=OP.mult)
            nc.vector.tensor_scalar_add(sz, sz, 1.0)
            biasT = biasp.tile([128, LB], F32, tag="biasT")
            nc.scalar.activation(biasT, sz, AF.Ln)
            nc.vector.tensor_tensor(biasT, biasT, msk, op=OP.add)
            biasTs[b] = biasT

            x8t, x2t = x8s[b], x2s[b]
            # row stats via fp8 DoubleRow matmuls; psum halves land on
            # partition blocks of [128, 8] tiles (l = 8p + a)
            murow = rows.tile([1, L], F32, tag="murow")
            sqrow = rows.tile([1, L], F32, tag="sqrow")
            for half in range(2):
                sl = slice(half * 512, (half + 1) * 512)
                mean_ps = scps.tile([1, 512], F32, tag="sc", name="mean_ps")
                sq_ps = scps.tile([1, 512], F32, tag="sc", name="sq_ps")
                for cp in range(CBP):
                    nc.tensor.matmul(mean_ps, ones8,
                                     x8t[:, 2 * cp:2 * cp + 2, sl],
                                     start=(cp == 0), stop=(cp == CBP - 1),
                                     perf_mode=DR)
                for cp in range(CBP):
                    nc.tensor.matmul(sq_ps, ones8,
                                     x2t[:, 2 * cp:2 * cp + 2, sl],
                                     start=(cp == 0), stop=(cp == CBP - 1),
                                     perf_mode=DR)
                nc.vector.tensor_scalar_mul(murow[0:1, sl], mean_ps,
                                            1.0 / (C * SX))
                nc.vector.tensor_scalar_mul(sqrow[0:1, sl], sq_ps, 1.0 / C)
            mu8 = rows.tile([128, 8], F32, tag="mu8")
            nc.sync.dma_start(out=mu8, in_=murow)
            var8 = rows.tile([128, 8], F32, tag="var8")
            nc.sync.dma_start(out=var8, in_=sqrow)
            t8 = rows.tile([128, 8], F32, tag="t8")
            nc.vector.tensor_tensor(t8, mu8, mu8, op=OP.mult)
            nc.vector.tensor_tensor(var8, var8, t8, op=OP.subtract)
            nc.scalar.activation(var8, var8, AF.Sqrt, bias=eps8)  # std
            r8f = rows.tile([128, 8], F32, tag="r8f")
            nc.vector.reciprocal(r8f, var8)
            nc.vector.tensor_tensor(t8, mu8, r8f, op=OP.mult)
            nc.vector.tensor_scalar_mul(t8, t8, -1.0)             # s = -mu*r
            # rows in [1, L] layout (l = 8p + a linearization)
            rfrow = rows.tile([1, L], F32, tag="rfrow")
            nc.sync.dma_start(out=rfrow, in_=r8f)
            sfrow = rows.tile([1, L], F32, tag="sfrow")
            nc.sync.dma_start(out=sfrow, in_=t8)
            # kh needs row-broadcasts (l on free dim)
            rxb = bcastp.tile([128, L], F32, tag="rxb")
            nc.gpsimd.partition_broadcast(rxb, rfrow)
            sxb = bcastp.tile([128, L], F32, tag="sxb")
            nc.gpsimd.partition_broadcast(sxb, sfrow)
            rxbs[b], sxbs[b] = rxb, sxb
            # vh needs per-l-block columns: rv8[p, lb] = r[128*lb + p].
            # The permutation runs through DRAM (SBUF-side APs can't express
            # it): r8f -> DRAM[L] is linear (l = 8p + a), then a 2D strided
            # read back.
            rfd = drp.tile([L], F32, tag="rfd")
            nc.scalar.dma_start(out=rfd, in_=r8f)
            sfd = drp.tile([L], F32, tag="sfd")
            nc.scalar.dma_start(out=sfd, in_=t8)
            rv8 = cols.tile([128, LB], F32, tag="rv8")
            nc.sync.dma_start(out=rv8,
                              in_=rfd.rearrange("(a p) -> p a", p=128))
            sv8 = cols.tile([128, LB], F32, tag="sv8")
            nc.sync.dma_start(out=sv8,
                              in_=sfd.rearrange("(a p) -> p a", p=128))
            rvs[b], svs[b] = rv8, sv8

            # lookahead loads last (recycle-semaphore parking discipline)
            if b + 1 < BL:
                x8s[b + 1] = x8p.tile([128, CB, L], FP8, tag="x8r",
                                      name=f"x8r_b{b + 1}")
                nc.sync.dma_start(out=x8s[b + 1],
                                  in_=x8r_d[b + 1].rearrange("a p l -> p a l"))
                x2s[b + 1] = x2p.tile([128, CB, L], FP8, tag="x28",
                                      name=f"x28_b{b + 1}")
                nc.sync.dma_start(out=x2s[b + 1],
                                  in_=x28_d[b + 1].rearrange("a p l -> p a l"))
            if b + 2 < BL:
                xns[b + 2] = xnp.tile([128, CB, L], BF16, tag="xn",
                                      name=f"xn_b{b + 2}")
                for cb in range(CB):
                    nc.gpsimd.dma_start(
                        out=xns[b + 2][:, cb, :],
                        in_=xT[b + 2, cb * 128:(cb + 1) * 128, :])

        def projections(b):
            xn = xns[b]
            x8t = x8s[b]
            rxb, sxb = rxbs[b], sxbs[b]
            rv8, sv8 = rvs[b], svs[b]
            # K projection fp8 DoubleRow on raw x8; LN fold on copy:
            #   kh = (kps * DESC) * r[l]  +  s[l] * colsumK[d]
            kh = khp.tile([HD, H, L], BF16, tag="kh")
            for h in range(H):
                kps0 = kvps.tile([128, 512], F32, tag="kv", name="kps0")
                kps1 = kvps.tile([128, 512], F32, tag="kv", name="kps1")
                for cp in range(CBP):
                    st, sp = (cp == 0), (cp == CBP - 1)
                    nc.tensor.matmul(kps0[:HD, :],
                                     wk8[:, cp, :, h * HD:(h + 1) * HD],
                                     x8t[:, 2 * cp:2 * cp + 2, 0:512],
                                     start=st, stop=sp, perf_mode=DR)
                    nc.tensor.matmul(kps1[:HD, :],
                                     wk8[:, cp, :, h * HD:(h + 1) * HD],
                                     x8t[:, 2 * cp:2 * cp + 2, 512:1024],
                                     start=st, stop=sp, perf_mode=DR)
                for lc, kps in ((0, kps0), (1, kps1)):
                    sl = slice(lc * 512, (lc + 1) * 512)
                    nc.vector.scalar_tensor_tensor(
                        kh[:, h, sl], kps[:HD, :], KH_DESCALE, rxb[0:96, sl],
                        op0=OP.mult, op1=OP.mult)
                    nc.vector.scalar_tensor_tensor(
                        kh[:, h, sl], sxb[0:96, sl], csk[:, h:h + 1],
                        kh[:, h, sl], op0=OP.mult, op1=OP.add)

            # V projection bf16 on raw x; LN fold on copy:
            #   vh = vps * r_col  +  s_col * colsumV[d]
            vh = vhp.tile([128, LB, H, HD + 1], BF16, tag="vh")
            nc.vector.tensor_copy(
                vh[:, :, :, HD:HD + 1],
                ones64.rearrange("p (a b c) -> p a b c", a=LB, b=H))
            for lb in range(LB):
                for dc in range(2):
                    dsl = slice(dc * 4 * HD, (dc + 1) * 4 * HD)
                    vps = kvps.tile([128, 512], F32, tag="kv")
                    for cb in range(CB):
                        nc.tensor.matmul(vps[:, :4 * HD],
                                         xn[:, cb, lb * 128:(lb + 1) * 128],
                                         wv[:, cb, dsl],
                                         start=(cb == 0), stop=(cb == CB - 1))
                    vsl = vh[:, lb, 4 * dc:4 * dc + 4, 0:HD]
                    nc.scalar.activation(vsl, vps[:, :4 * HD], AF.Copy,
                                         scale=rv8[:, lb:lb + 1])
                    nc.vector.scalar_tensor_tensor(
                        vsl, csvb[:, dsl], sv8[:, lb:lb + 1], vsl,
                        op0=OP.mult, op1=OP.add)
            return kh, vh

        khs = [None] * BL
        vhs = [None] * BL
        serows = [None] * BL
        se8s = [None] * BL
        otss = [None] * BL

        def att_core(b):
            kh, vh = khs[b], vhs[b]
            biasT = biasTs[b]
            serow = recipp.tile([1, H * Q], F32, tag="serow", bufs=1)
            ots = [None] * H
            for hp in range(H // 2):
                h0, h1 = 2 * hp, 2 * hp + 1
                av0 = avps.tile([HD + 1, Q], F32, tag="av", name=f"av{h0}")
                av1 = avps.tile([HD + 1, Q], F32, tag="av", name=f"av{h1}")
                for lb in range(LB):
                    sc = scps.tile([128, 2, Q], F32, tag="sc")
                    nc.tensor.matmul(sc[:, 0, :],
                                     kh[:, h0, lb * 128:(lb + 1) * 128],
                                     qhT[:, h0, :], start=True, stop=True)
                    nc.tensor.matmul(sc[:, 1, :],
                                     kh[:, h1, lb * 128:(lb + 1) * 128],
                                     qhT[:, h1, :], start=True, stop=True)
                    ex = expp.tile([128, 2, Q], BF16, tag="ex")
                    nc.scalar.activation(ex, sc, AF.Exp,
                                         bias=biasT[:, lb:lb + 1])
                    nc.tensor.matmul(av0, vh[:, lb, h0, :], ex[:, 0, :],
                                     start=(lb == 0), stop=(lb == LB - 1))
                    nc.tensor.matmul(av1, vh[:, lb, h1, :], ex[:, 1, :],
                                     start=(lb == 0), stop=(lb == LB - 1))
                for h, av in ((h0, av0), (h1, av1)):
                    nc.vector.tensor_copy(serow[0:1, h * Q:(h + 1) * Q],
                                          av[HD:HD + 1, :])
                    ot = outtp.tile([HD, Q], BF16, tag="ot", name=f"ot{h}")
                    nc.scalar.copy(ot, av[0:HD, :])
                    ots[h] = ot
            se8 = recipp.tile([128, H * Q // 128], F32, tag="se8")
            nc.scalar.dma_start(out=se8, in_=serow)
            serows[b], se8s[b], otss[b] = serow, se8, ots

        def att_finish(b):
            se8, ots = se8s[b], otss[b]
            nc.vector.reciprocal(se8, se8)
            se8b = recipp.tile([128, H * Q // 128], BF16, tag="se8b")
            nc.vector.tensor_copy(se8b, se8)
            sed = drp.tile([H * Q], BF16, tag="sed")
            nc.scalar.dma_start(out=sed, in_=se8b)
            rball = recipp.tile([HD, H, Q], BF16, tag="rball", bufs=1)
            nc.scalar.dma_start(out=rball.rearrange("p a q -> p (a q)"),
                                in_=bcast_dram(sed, HD, H * Q))
            otbs = []
            for h in range(H):
                nc.vector.tensor_tensor(ots[h], ots[h], rball[:, h, :],
                                        op=OP.mult)
                otbs.append(ots[h])

            # out projection: final[q, dm] = sum_h outT_h.T @ WoT_h  (+bo2)
            for qb in range(QB):
                fin = finp.tile([128, D], F32, tag="fin")
                for dc, dn in ((0, 512), (512, 256)):
                    fps = fips.tile([128, 2, Q], F32, tag="sc", name="fps")
                    fpsv = fps.rearrange("p a q -> p (a q)")
                    for h in range(H):
                        nc.tensor.matmul(fpsv[:, :dn],
                                         otbs[h][:, qb * 128:(qb + 1) * 128],
                                         wo[:, h, dc:dc + dn],
                                         start=(h == 0), stop=(h == H - 1))
                    nc.vector.tensor_tensor(fin[:, dc:dc + dn], fpsv[:, :dn],
                                            bob[:, dc:dc + dn], op=OP.add)
                nc.scalar.dma_start(out=out_d[b, qb * 128:(qb + 1) * 128, :],
                                     in_=fin)

        # ---- software-pipelined schedule ----
        front_end(0)
        khs[0], vhs[0] = projections(0)
        front_end(1)
        for b in range(BL):
            att_core(b)
            if b + 1 < BL:
                khs[b + 1], vhs[b + 1] = projections(b + 1)
            att_finish(b)
            if b + 2 < BL:
                front_end(b + 2)

    nc.compile()
    return nc


_CACHE = {}


def make_in_maps(inputs):
    import ml_dtypes
    bf16 = ml_dtypes.bfloat16
    f8 = ml_dtypes.float8_e4m3

    x = np.ascontiguousarray(inputs["x"], dtype=np.float32)
    size = np.asarray(inputs["size"], dtype=np.float32)
    mask = np.asarray(inputs["attention_mask"], dtype=np.float32)
    query = np.asarray(inputs["query"], dtype=np.float32)
    lnqw = np.asarray(inputs["ln_q_w"], np.float32)
    lnqb = np.asarray(inputs["ln_q_b"], np.float32)
    lnkw = np.asarray(inputs["ln_k_w"], np.float32)
    lnkb = np.asarray(inputs["ln_k_b"], np.float32)
    Wq = np.asarray(inputs["Wq"], np.float32)
    Wk = np.asarray(inputs["Wk"], np.float32)
    Wv = np.asarray(inputs["Wv"], np.float32)
    Wo = np.asarray(inputs["Wo"], np.float32)

    xb = x.astype(bf16)
    xT = np.ascontiguousarray(xb.transpose(0, 2, 1))       # [B, C, L] bf16
    xf = xb.astype(np.float32)
    # fp8 streams, layout [B, CB, 128, L] (c = cb*128 + p)
    x8r = np.ascontiguousarray(
        (xf.transpose(0, 2, 1) * SX).reshape(B, CB, 128, L).astype(f8))
    x28 = np.ascontiguousarray(
        (xf.transpose(0, 2, 1) ** 2).reshape(B, CB, 128, L).astype(f8))

    # host query path in f32
    qmu = query.mean(-1, keepdims=True)
    qvar = query.var(-1, keepdims=True)
    qln = (query - qmu) / np.sqrt(qvar + EPS) * lnqw + lnqb
    qh = (qln @ Wq.T + np.asarray(inputs["bq"], np.float32)) * SCALE  # [Q, D]
    qhT_hm = np.ascontiguousarray(
        qh.reshape(Q, H, HD).transpose(2, 1, 0).astype(bf16))  # [HD, H, Q]

    # folded weights and LN-fold constants
    wkf = (Wk * lnkw[None, :]).T                            # [C, D]
    wvf = (Wv * lnkw[None, :]).T
    Wk8 = np.ascontiguousarray(
        (wkf * SW).reshape(CBP, 2, 128, D).transpose(2, 0, 1, 3).astype(f8))
    WvT = np.ascontiguousarray(wvf.astype(bf16))
    csK = wkf.sum(0)                                        # [D]
    csK_hm = np.ascontiguousarray(
        csK.reshape(H, HD).T.astype(np.float32))            # [HD, H]
    csV = wvf.sum(0).astype(np.float32)
    bv_f = np.asarray(inputs["bv"], np.float32) + lnkb @ Wv.T
    bo2 = (np.asarray(inputs["bo"], np.float32) + bv_f @ Wo.T)
    WoT = np.ascontiguousarray(
        Wo.T.reshape(H, HD, D).transpose(1, 0, 2).astype(bf16))

    size2 = np.ascontiguousarray(size[:, :, 0])            # [B, L]
    mask2 = np.ascontiguousarray(mask[:, 0, :])            # [B, L]
    szmk = np.ascontiguousarray(np.concatenate(
        [size2.reshape(B, LB, 128).transpose(0, 2, 1),
         mask2.reshape(B, LB, 128).transpose(0, 2, 1)], axis=2))

    common = {
        "qhT_hm": qhT_hm, "Wk8": Wk8, "WvT": WvT, "WoT": WoT,
        "csK_hm": csK_hm, "csV": csV, "bo2": bo2.astype(np.float32),
    }
    in_maps = []
    for i in range(N_CORES):
        sl = slice(i * BL, (i + 1) * BL)
        m = dict(common)
        m["xT"] = np.ascontiguousarray(xT[sl])
        m["x8r"] = np.ascontiguousarray(x8r[sl])
        m["x28"] = np.ascontiguousarray(x28[sl])
        m["szmk"] = np.ascontiguousarray(szmk[sl])
        in_maps.append(m)

    return in_maps


def kernel(**inputs):
    in_maps = make_in_maps(inputs)
    if "nc" not in _CACHE:
        _CACHE["nc"] = build_program()
    nc = _CACHE["nc"]

    for attempt in range(3):
        res = bass_utils.run_bass_kernel_spmd(nc, in_maps,
                                              core_ids=list(range(N_CORES)))
        out = np.concatenate([res.results[i]["out"] for i in range(N_CORES)],
                             axis=0)
        if np.isfinite(out).all():
            return out
    return out
